# revision 1
# baseline (speedup 1.0000x reference)
"""Trainium2 Bass kernel for nn_AttentionLayer_77558519431766.

Math: the reference computes softmax over a size-1 axis, which is
identically 1.0, so the attention MLP is dead code and

    out[b, e] = sum_{i<j} x[b,i,e] * x[b,j,e]
              = 0.5 * ((sum_f x[b,f,e])^2 - sum_f x[b,f,e]^2)

Implementation (per 128-sample chunk, layout [128b, f*64+e]):
  1. ACT casts x to bf16.
  2. PE transposes each [128b, 128(f2,e)] block into PSUM, so pairs of
     f-rows land on partitions.
  3. DVE copies the transposed blocks back to SBUF (for s), ACT squares
     them into SBUF (for q).
  4. PE runs two matmul accumulation chains against a stacked-identity
     mask [128,64] (row (f2,e) is one-hot at e), yielding
     s = sum_f x and q = sum_f x^2 as [128b, 64e] in PSUM.
  5. res = 0.5*s^2 - 0.5*q, DMA out.

Sharding: pure data parallelism, batch 2048 -> 8 shards of 256.
"""

import numpy as np

try:
    import concourse.bass as bass  # noqa: F401
except ImportError:  # pragma: no cover
    import sys

    sys.path.insert(0, "/opt/trn_rl_repo")

_B, _F, _E = 2048, 50, 64
_NCORES = 8
_BS = _B // _NCORES  # 256 rows per core
_ROW = _F * _E  # 3200 floats per row
_P = 128  # SBUF partitions
_NBLK = _ROW // _P  # 25 transpose blocks per chunk


def _make_tc_class():
    """TileContext with a slim kernel tail.

    Stock TileContext ends with drain -> full all-engine barrier ->
    semaphore clear -> second full barrier (~6-8us of EVSEM butterfly).
    The Bass preamble already dma_reset+sem_clears the entire kernel
    semaphore range at the start of every execution, so the tail clear
    and second barrier are redundant for a single-TileContext kernel.
    Keep the global-clock drain (output DMA completion) plus one cheap
    sequencer-level barrier.
    """
    from concourse.tile import TileContext
    from concourse.vector_clock import ScopedClock

    class SlimTailTileContext(TileContext):
        def _drain_and_barrier(self, tick_clock, wait_clock):
            drain_inst = self.nc.sync.drain()
            wait_clock.add_sem_waits(
                drain_inst.ins, ScopedClock({None: tick_clock.global_clock})
            )
            self.nc.all_engine_barrier(sem_only=True)
            popped = self.nc._tile_sem_poison_stack.pop()
            assert popped is self._sem_poison

    return SlimTailTileContext


def _build():
    import concourse.bacc as bacc
    import concourse.mybir as mybir

    TileContext = _make_tc_class()

    f32 = mybir.dt.float32
    bf16 = mybir.dt.bfloat16
    SQ = mybir.ActivationFunctionType.Square
    CP = mybir.ActivationFunctionType.Copy
    ALU = mybir.AluOpType
    HALF_SQRT = float(np.float32(np.sqrt(0.5)))

    i32 = mybir.dt.int32

    nc = bacc.Bacc()
    x = nc.declare_dram_parameter("inputs", [_BS, _ROW], f32, isOutput=False)
    out = nc.declare_dram_parameter("out", [_BS, _E], f32, isOutput=True)

    n_chunks = _BS // _P  # 2
    halves = [(0, _NBLK)]  # full-width DMA keeps 12.8KB/row packets (345 GB/s)
    # transpose blocks per PSUM tile (<= 1 bank bf16); smallest group LAST
    # so the final cast/copy/square hop on the critical tail is shortest
    groups = [7, 7, 7, 4]

    with TileContext(nc) as tc:
        with (
            tc.tile_pool(name="consts", bufs=1) as cpool,
            tc.tile_pool(name="x", bufs=4) as xpool,
            tc.tile_pool(name="xb", bufs=4) as xbpool,
            tc.tile_pool(name="xT", bufs=4) as xtpool,
            tc.tile_pool(name="xsq", bufs=4) as sqpool,
            tc.tile_pool(name="pt", bufs=3, space="PSUM") as ptpool,
            tc.tile_pool(name="acc", bufs=2, space="PSUM") as accpool,
            tc.tile_pool(name="small", bufs=2) as spool,
        ):
            # Warm op: forces the ACT function-table load off the critical
            # path (it otherwise lands right before the first Square, after
            # a cross-engine wait).
            warm = spool.tile([_P, 1], f32, tag="warm")
            nc.gpsimd.memset(warm[:], 0.0)
            nc.scalar.activation(warm[:], warm[:], SQ)

            # Constants built on-chip (a DMA for these queues behind the
            # input packets and stalls the first transposes by multiple us).
            # iota with channel_multiplier=-1 gives v[p,j] = j - p, so
            # identity = (v == 0); the stacked mask [128,64] has ones where
            # j - p is 0 or -64.
            iot_i = cpool.tile([_P, _P], i32, tag="iot_i")
            iot_m = cpool.tile([_P, _E], i32, tag="iot_m")
            ident = cpool.tile([_P, _P], bf16, tag="ident")
            mask = cpool.tile([_P, _E], bf16, tag="mask")
            mask_b = cpool.tile([_P, _E], bf16, tag="mask_b")
            nc.gpsimd.iota(iot_i[:], pattern=[[1, _P]], base=0, channel_multiplier=-1)
            nc.gpsimd.iota(iot_m[:], pattern=[[1, _E]], base=0, channel_multiplier=-1)
            nc.vector.tensor_scalar(
                ident[:], iot_i[:], 0, None, op0=ALU.is_equal
            )
            nc.vector.tensor_scalar(
                mask[:], iot_m[:], 0, None, op0=ALU.is_equal
            )
            nc.vector.tensor_scalar(
                mask_b[:], iot_m[:], -_E, None, op0=ALU.is_equal
            )
            nc.vector.tensor_add(mask[:], mask[:], mask_b[:])
            # Pre-scale the chain masks so the combine needs no scaling:
            # s-chain mask = sqrt(0.5)*one-hot -> s'^2 = 0.5*s^2 (up to the
            # bf16 rounding of sqrt(0.5): (c^2-0.5)*s^2 ~ 1e-4 rel, noise
            # next to the 2.4e-3 bf16-squares error); q-chain mask =
            # 0.5*one-hot (exact in bf16) -> q' = 0.5*q.
            maskh = cpool.tile([_P, _E], bf16, tag="maskh")
            maskq = cpool.tile([_P, _E], bf16, tag="maskq")
            nc.vector.tensor_scalar_mul(maskh[:], mask[:], HALF_SQRT)
            nc.vector.tensor_scalar_mul(maskq[:], mask[:], 0.5)

            for c in range(n_chunks):
                rows = slice(c * _P, (c + 1) * _P)
                # separate banks: a start=True matmul clears its whole bank,
                # so the two accumulation chains must not share one
                s_t = accpool.tile([_P, _E], f32, tag="s")
                q_t = accpool.tile([_P, _E], f32, tag="q")
                s_ps = s_t[:]
                q_ps = q_t[:]
                for blk0, nblk in halves:
                    cols = slice(blk0 * _P, (blk0 + nblk) * _P)
                    n = nblk * _P
                    xt = xpool.tile([_P, n], f32, tag="x")
                    nc.sync.dma_start(out=xt[:], in_=x[rows, cols])
                    xbt = xbpool.tile([_P, n], bf16, tag="xb")

                    xT = xtpool.tile([_P, n], bf16, tag="xT")
                    xsq = sqpool.tile([_P, n], bf16, tag="xsq")
                    g0 = 0
                    for gn in groups:
                        gcols = slice(g0 * _P, (g0 + gn) * _P)
                        # per-group cast: lets the tail chunk pipeline at
                        # group granularity (gpsimd CAST measured 4x slower
                        # than DVE's 2x mode, so this stays on DVE)
                        nc.vector.tensor_copy(xbt[:, gcols], xt[:, gcols])
                        pt = ptpool.tile([_P, groups[0] * _P], bf16, tag="pt")
                        for j in range(gn):
                            k = g0 + j
                            nc.tensor.transpose(
                                pt[:, j * _P : (j + 1) * _P],
                                xbt[:, k * _P : (k + 1) * _P],
                                ident[:],
                            )
                        nc.vector.tensor_copy(xT[:, gcols], pt[:, : gn * _P])
                        nc.scalar.activation(xsq[:, gcols], pt[:, : gn * _P], SQ)
                        g0 += gn
                    for k in range(nblk):
                        kk = blk0 + k
                        bcols = slice(k * _P, (k + 1) * _P)
                        nc.tensor.matmul(
                            s_ps,
                            xT[:, bcols],
                            maskh[:],
                            start=(kk == 0),
                            stop=(kk == _NBLK - 1),
                        )
                        nc.tensor.matmul(
                            q_ps,
                            xsq[:, bcols],
                            maskq[:],
                            start=(kk == 0),
                            stop=(kk == _NBLK - 1),
                        )

                # res = s'^2 - q' = 0.5*s^2 - 0.5*q
                m2 = spool.tile([_P, _E], f32, tag="m2")
                res = spool.tile([_P, _E], f32, tag="res")
                nc.scalar.activation(m2[:], s_ps, SQ)
                nc.vector.tensor_sub(res[:], m2[:], q_ps)
                nc.sync.dma_start(out=out[rows, :], in_=res[:])
    nc.compile()
    return nc


_WALRUS_EXTRA = []


def _patch_walrus():
    """Cap walrus's semaphore allocation: the NEFF postamble zeroes every
    allocated semaphore one event-sem op at a time (spaced to dodge the
    event-accel erratum), so unused semaphores cost ~150ns each at the
    kernel tail."""
    from concourse import bass_utils

    if getattr(bass_utils, "_walrus_patched", False):
        return
    real_run = bass_utils.run_command

    def run2(cmd, **kw):
        if cmd and "walrus_driver" in str(cmd[0]):
            cmd = list(cmd) + _WALRUS_EXTRA
        return real_run(cmd, **kw)

    bass_utils.run_command = run2
    bass_utils._walrus_patched = True


def _run(in_maps, **kwargs):
    from concourse.bass_utils import run_bass_kernel_spmd

    _patch_walrus()
    nc = _build()
    return run_bass_kernel_spmd(nc, in_maps, core_ids=list(range(_NCORES)), **kwargs)


def _shard(inputs: np.ndarray):
    x = np.ascontiguousarray(
        np.asarray(inputs, dtype=np.float32).reshape(_B, _ROW)
    )
    return [
        {"inputs": np.ascontiguousarray(x[i * _BS : (i + 1) * _BS])}
        for i in range(_NCORES)
    ]


def kernel(
    inputs: np.ndarray,
    weight_attention: np.ndarray = None,
    weight_projection: np.ndarray = None,
    weight_bias: np.ndarray = None,
) -> np.ndarray:
    # weights are dead code (softmax over a size-1 axis == 1.0)
    res = _run(_shard(inputs))
    return np.concatenate([r["out"] for r in res.results], axis=0)



# revision 4
# speedup vs baseline: 1.0289x; 1.0289x over previous
"""Trainium2 Bass kernel for nn_AttentionLayer_77558519431766.

Math: the reference computes softmax over a size-1 axis, which is
identically 1.0, so the attention MLP is dead code and

    out[b, e] = sum_{i<j} x[b,i,e] * x[b,j,e]
              = 0.5 * ((sum_f x[b,f,e])^2 - sum_f x[b,f,e]^2)

Design (v2, per 128-sample chunk, layout [128b, f*64+e]):
  - Input DMA in 4 column slices per chunk so compute streams behind
    the DMA instead of waiting for the full 1.6MB chunk. All input
    DMAs are issued before anything else queues on the sync engine
    (a chunk-0 output DMA issued earlier would head-of-line block
    chunk 1's input issues).
  - DVE casts each slice to bf16, then computes s = sum_f x with an
    in-layout pairwise tree (contiguous halves stay 64-col aligned),
    so the s-path never touches PE or PSUM.
  - PE transposes 128-col blocks (2 fields) into PSUM; ACT squares
    them back to SBUF with scale sqrt(0.5) (-> 0.5*x^2); PE runs one
    mask-stationary accumulation chain for q = sum_f 0.5*x^2.
  - res = 0.5*s^2 - q' via one ACT square + one DVE subtract.
  - PE warm-up dummies run during the preamble/DMA fill so the HAM
    clock gate (1.2GHz cold -> 2.4GHz warm) lifts before real work.

Sharding: pure data parallelism, batch 2048 -> 8 shards of 256.
"""

import numpy as np

try:
    import concourse.bass as bass  # noqa: F401
except ImportError:  # pragma: no cover
    import sys

    sys.path.insert(0, "/opt/trn_rl_repo")

_B, _F, _E = 2048, 50, 64
_NCORES = 8
_BS = _B // _NCORES  # 256 rows per core
_ROW = _F * _E  # 3200 floats per row
_P = 128  # SBUF partitions

# DMA column slices per chunk: (col0, col1, n_fields)
_SLICES = [(0, 1280, 20), (1280, 2560, 20), (2560, 3072, 8), (3072, 3200, 2)]
# transpose/square groups: (block0, nblocks); 128-col blocks, <=5 per PSUM tile
_TGROUPS = [(0, 5), (5, 10), (10, 15), (15, 20), (20, 24), (24, 25)]
_NBLK = 25
_NWARM = 32  # PE warm-up dummies (~32*107ns = 3.4us of PE activity)


def _make_tc_class():
    """TileContext with a slim kernel tail (see baseline notes: the Bass
    preamble already resets semaphores each execution, so the stock tail
    clear + second barrier are redundant)."""
    from concourse.tile import TileContext
    from concourse.vector_clock import ScopedClock

    class SlimTailTileContext(TileContext):
        def _drain_and_barrier(self, tick_clock, wait_clock):
            drain_inst = self.nc.sync.drain()
            wait_clock.add_sem_waits(
                drain_inst.ins, ScopedClock({None: tick_clock.global_clock})
            )
            self.nc.all_engine_barrier(sem_only=True)
            popped = self.nc._tile_sem_poison_stack.pop()
            assert popped is self._sem_poison

    return SlimTailTileContext


def _emit_tree(nc, xb, c0, nf, out_ap, sc):
    """Sum nf contiguous 64-col fields of xb starting at col c0 into
    out_ap [128, 64] (f32) via pairwise halving adds (bf16 temps in sc)."""
    add = nc.vector.tensor_add
    if nf == 2:
        add(out_ap, xb[:, c0 : c0 + 64], xb[:, c0 + 64 : c0 + 128])
        return
    if nf == 8:
        add(sc[:, 0:256], xb[:, c0 : c0 + 256], xb[:, c0 + 256 : c0 + 512])
        add(sc[:, 256:384], sc[:, 0:128], sc[:, 128:256])
        add(out_ap, sc[:, 256:320], sc[:, 320:384])
        return
    assert nf == 20
    add(sc[:, 0:640], xb[:, c0 : c0 + 640], xb[:, c0 + 640 : c0 + 1280])
    add(sc[:, 640:960], sc[:, 0:320], sc[:, 320:640])
    add(sc[:, 960:1088], sc[:, 640:768], sc[:, 768:896])
    add(sc[:, 1088:1152], sc[:, 960:1024], sc[:, 1024:1088])
    add(out_ap, sc[:, 1088:1152], sc[:, 896:960])


def _build():
    import concourse.bacc as bacc
    import concourse.mybir as mybir

    TileContext = _make_tc_class()

    f32 = mybir.dt.float32
    bf16 = mybir.dt.bfloat16
    i32 = mybir.dt.int32
    SQ = mybir.ActivationFunctionType.Square
    ALU = mybir.AluOpType
    HALF_SQRT = float(np.float32(np.sqrt(0.5)))

    nc = bacc.Bacc()
    x = nc.declare_dram_parameter("inputs", [_BS, _ROW], f32, isOutput=False)
    out = nc.declare_dram_parameter("out", [_BS, _E], f32, isOutput=True)

    n_chunks = _BS // _P  # 2

    with TileContext(nc) as tc:
        with (
            tc.tile_pool(name="consts", bufs=1) as cpool,
            tc.tile_pool(name="xt", bufs=2) as xtpool,
            tc.tile_pool(name="xb", bufs=2) as xbpool,
            tc.tile_pool(name="xsq", bufs=2) as sqpool,
            tc.tile_pool(name="tree", bufs=2) as trpool,
            tc.tile_pool(name="sp", bufs=2) as sppool,
            tc.tile_pool(name="pt", bufs=3, space="PSUM") as ptpool,
            tc.tile_pool(name="acc", bufs=2, space="PSUM") as accpool,
            tc.tile_pool(name="wp", bufs=2, space="PSUM") as wppool,
            tc.tile_pool(name="small", bufs=2) as spool,
        ):
            # Warm op: forces the ACT function-table load early (off the
            # critical path).
            warm = spool.tile([_P, 1], f32, tag="warm")
            nc.gpsimd.memset(warm[:], 0.0)
            nc.scalar.activation(warm[:], warm[:], SQ)

            # Constants on-chip: identity [128,128] and the stacked two-hot
            # mask [128,64] (row (f2,e) -> col e for both field copies).
            iot_i = cpool.tile([_P, _P], i32, tag="iot_i")
            iot_m = cpool.tile([_P, _E], i32, tag="iot_m")
            ident = cpool.tile([_P, _P], bf16, tag="ident")
            mask = cpool.tile([_P, _E], bf16, tag="mask")
            mask_b = cpool.tile([_P, _E], bf16, tag="mask_b")
            nc.gpsimd.iota(iot_i[:], pattern=[[1, _P]], base=0, channel_multiplier=-1)
            nc.gpsimd.iota(iot_m[:], pattern=[[1, _E]], base=0, channel_multiplier=-1)
            nc.vector.tensor_scalar(ident[:], iot_i[:], 0, None, op0=ALU.is_equal)
            nc.vector.tensor_scalar(mask[:], iot_m[:], 0, None, op0=ALU.is_equal)
            nc.vector.tensor_scalar(mask_b[:], iot_m[:], -_E, None, op0=ALU.is_equal)
            nc.vector.tensor_add(mask[:], mask[:], mask_b[:])

            # PE warm-up: keep the PE busy during the preamble/DMA fill so
            # the HAM clock gate lifts to 2.4GHz before the real transposes.
            for i in range(_NWARM):
                wp = wppool.tile([_P, _P], bf16, tag="wp")
                nc.tensor.transpose(wp[:], ident[:], ident[:])

            # Phase A: all input DMAs (sync engine) for both chunks.
            xts, xbs, xsqs, sps, qts = [], [], [], [], []
            for c in range(n_chunks):
                rows = slice(c * _P, (c + 1) * _P)
                xt = xtpool.tile([_P, _ROW], f32, tag="xt")
                xb = xbpool.tile([_P, _ROW], bf16, tag="xb")
                xsq = sqpool.tile([_P, _ROW], bf16, tag="xsq")
                sp = sppool.tile([_P, len(_SLICES) * _E], f32, tag="sp")
                q_t = accpool.tile([_P, _E], f32, tag="q")
                xts.append(xt)
                xbs.append(xb)
                xsqs.append(xsq)
                sps.append(sp)
                qts.append(q_t)
                for c0, c1, nf in _SLICES:
                    nc.sync.dma_start(out=xt[:, c0:c1], in_=x[rows, c0:c1])

            # Phase B: DVE casts + s-trees per slice, both chunks.
            for c in range(n_chunks):
                xt, xb, sparts = xts[c], xbs[c], sps[c]
                for si, (c0, c1, nf) in enumerate(_SLICES):
                    nc.vector.tensor_copy(xb[:, c0:c1], xt[:, c0:c1])
                    sc = trpool.tile([_P, 1152], bf16, tag="tr")
                    _emit_tree(
                        nc, xb, c0, nf,
                        sparts[:, si * _E : (si + 1) * _E], sc,
                    )

            # Phase C: PE transposes + ACT squares + q-chains + combine,
            # software-pipelined by one group so PE never stalls on ACT.
            for c in range(n_chunks):
                rows = slice(c * _P, (c + 1) * _P)
                xb, xsq, sparts = xbs[c], xsqs[c], sps[c]
                q_ps = qts[c][:]

                def emit_group_mms(g):
                    b0, b1 = _TGROUPS[g]
                    for k in range(b0, b1):
                        nc.tensor.matmul(
                            q_ps,
                            xsq[:, k * _P : (k + 1) * _P],
                            mask[:],
                            start=(k == 0),
                            stop=(k == _NBLK - 1),
                        )

                for g, (b0, b1) in enumerate(_TGROUPS):
                    gw = (b1 - b0) * _P
                    pt = ptpool.tile([_P, 5 * _P], bf16, tag="pt")
                    for j, k in enumerate(range(b0, b1)):
                        nc.tensor.transpose(
                            pt[:, j * _P : (j + 1) * _P],
                            xb[:, k * _P : (k + 1) * _P],
                            ident[:],
                        )
                    nc.scalar.activation(
                        xsq[:, b0 * _P : b1 * _P], pt[:, :gw], SQ, scale=HALF_SQRT
                    )
                    if g > 0:
                        emit_group_mms(g - 1)
                emit_group_mms(len(_TGROUPS) - 1)

                # s = sum of the 4 slice partials; res = 0.5*s^2 - q'
                v = spool.tile([_P, 2 * _E], f32, tag="v")
                s_t = spool.tile([_P, _E], f32, tag="s")
                m2 = spool.tile([_P, _E], f32, tag="m2")
                res = spool.tile([_P, _E], f32, tag="res")
                nc.vector.tensor_add(v[:], sparts[:, 0:128], sparts[:, 128:256])
                nc.vector.tensor_add(s_t[:], v[:, 0:64], v[:, 64:128])
                nc.scalar.activation(m2[:], s_t[:], SQ, scale=HALF_SQRT)
                nc.vector.tensor_sub(res[:], m2[:], q_ps)
                nc.sync.dma_start(out=out[rows, :], in_=res[:])
    nc.compile()
    return nc


_WALRUS_EXTRA = []


def _patch_walrus():
    """Cap walrus's semaphore allocation (unused semaphores cost ~150ns
    each in the NEFF postamble)."""
    from concourse import bass_utils

    if getattr(bass_utils, "_walrus_patched", False):
        return
    real_run = bass_utils.run_command

    def run2(cmd, **kw):
        if cmd and "walrus_driver" in str(cmd[0]):
            cmd = list(cmd) + _WALRUS_EXTRA
        return real_run(cmd, **kw)

    bass_utils.run_command = run2
    bass_utils._walrus_patched = True


def _run(in_maps, **kwargs):
    from concourse.bass_utils import run_bass_kernel_spmd

    _patch_walrus()
    nc = _build()
    return run_bass_kernel_spmd(nc, in_maps, core_ids=list(range(_NCORES)), **kwargs)


def _shard(inputs: np.ndarray):
    x = np.ascontiguousarray(
        np.asarray(inputs, dtype=np.float32).reshape(_B, _ROW)
    )
    return [
        {"inputs": np.ascontiguousarray(x[i * _BS : (i + 1) * _BS])}
        for i in range(_NCORES)
    ]


def kernel(
    inputs: np.ndarray,
    weight_attention: np.ndarray = None,
    weight_projection: np.ndarray = None,
    weight_bias: np.ndarray = None,
) -> np.ndarray:
    # weights are dead code (softmax over a size-1 axis == 1.0)
    res = _run(_shard(inputs))
    return np.concatenate([r["out"] for r in res.results], axis=0)


# revision 5
# speedup vs baseline: 1.0685x; 1.0385x over previous
"""Trainium2 Bass kernel for nn_AttentionLayer_77558519431766.

Math: the reference computes softmax over a size-1 axis, which is
identically 1.0, so the attention MLP is dead code and

    out[b, e] = sum_{i<j} x[b,i,e] * x[b,j,e]
              = 0.5 * ((sum_f x[b,f,e])^2 - sum_f x[b,f,e]^2)

Design (v3, per 128-sample chunk, layout [128b, f*64+e]):
  - Input arrives via SWDGE cast-DMA (gpsimd): f32 in HBM -> bf16 in
    SBUF, two 1600-col slices per chunk, so no on-chip cast is needed.
  - DVE computes s = sum_f x with an in-layout pairwise halving tree
    (6 big ops per 25-field slice; boundaries stay 64-col aligned).
  - PE transposes 128-col blocks (2 fields) into PSUM; ACT squares
    them back to SBUF with scale sqrt(0.5) (-> 0.5*x^2); PE runs a
    mask-stationary accumulation chain for q = sum_f 0.5*x^2.
  - res = 0.5*s^2 - q' via one ACT square + one DVE subtract.
  - Wide PE dummy matmuls (512-col moving) run during the DMA fill so
    the HAM clock gate (1.2GHz cold -> 2.4GHz warm) lifts early.

Sharding: pure data parallelism, batch 2048 -> 8 shards of 256.
"""

import numpy as np

try:
    import concourse.bass as bass  # noqa: F401
except ImportError:  # pragma: no cover
    import sys

    sys.path.insert(0, "/opt/trn_rl_repo")

_B, _F, _E = 2048, 50, 64
_NCORES = 8
_BS = _B // _NCORES  # 256 rows per core
_ROW = _F * _E  # 3200 floats per row
_P = 128  # SBUF partitions

_SLICES = [(0, 1600), (1600, 3200)]  # 25 fields each
_TGROUPS = [(0, 5), (5, 10), (10, 15), (15, 20), (20, 25)]
_NBLK = 25
_NWARM = 7  # wide PE warm-up matmuls (512 cols each, ~0.43us cold apiece)
_CAST_DMA = True  # SWDGE f32->bf16 during DMA; False = HWDGE + DVE cast


def _make_tc_class():
    """TileContext with a slim kernel tail (the Bass preamble already
    resets semaphores each execution, so the stock tail clear + second
    barrier are redundant)."""
    from concourse.tile import TileContext
    from concourse.vector_clock import ScopedClock

    class SlimTailTileContext(TileContext):
        def _drain_and_barrier(self, tick_clock, wait_clock):
            drain_inst = self.nc.sync.drain()
            wait_clock.add_sem_waits(
                drain_inst.ins, ScopedClock({None: tick_clock.global_clock})
            )
            self.nc.all_engine_barrier(sem_only=True)
            popped = self.nc._tile_sem_poison_stack.pop()
            assert popped is self._sem_poison

    return SlimTailTileContext


def _emit_tree(nc, xb, c0, out_ap, sc):
    """Sum 25 contiguous 64-col fields of xb starting at col c0 into
    out_ap [128, 64] via pairwise halving adds (bf16 temps in sc)."""
    add = nc.vector.tensor_add
    X = lambda a, b: xb[:, c0 + a : c0 + b]
    add(sc[:, 0:768], X(0, 768), X(768, 1536))        # 12 fields
    add(sc[:, 768:1152], sc[:, 0:384], sc[:, 384:768])    # 6
    add(sc[:, 1152:1344], sc[:, 768:960], sc[:, 960:1152])  # 3
    add(sc[:, 1344:1408], sc[:, 1152:1216], sc[:, 1216:1280])  # 1 (+left)
    add(sc[:, 1408:1472], sc[:, 1344:1408], sc[:, 1280:1344])
    add(out_ap, sc[:, 1408:1472], X(1536, 1600))      # + 25th field


def _build():
    import concourse.bacc as bacc
    import concourse.mybir as mybir

    TileContext = _make_tc_class()

    f32 = mybir.dt.float32
    bf16 = mybir.dt.bfloat16
    i32 = mybir.dt.int32
    SQ = mybir.ActivationFunctionType.Square
    ALU = mybir.AluOpType
    HALF_SQRT = float(np.float32(np.sqrt(0.5)))

    nc = bacc.Bacc()
    x = nc.declare_dram_parameter("inputs", [_BS, _ROW], f32, isOutput=False)
    out = nc.declare_dram_parameter("out", [_BS, _E], f32, isOutput=True)

    n_chunks = _BS // _P  # 2

    with TileContext(nc) as tc:
        with (
            tc.tile_pool(name="consts", bufs=1) as cpool,
            tc.tile_pool(name="xb", bufs=2) as xbpool,
            tc.tile_pool(name="xt", bufs=2) as xtpool,
            tc.tile_pool(name="xsq", bufs=2) as sqpool,
            tc.tile_pool(name="tree", bufs=2) as trpool,
            tc.tile_pool(name="sp", bufs=2) as sppool,
            tc.tile_pool(name="pt", bufs=3, space="PSUM") as ptpool,
            tc.tile_pool(name="acc", bufs=2, space="PSUM") as accpool,
            tc.tile_pool(name="wp", bufs=1, space="PSUM") as wppool,
            tc.tile_pool(name="small", bufs=2) as spool,
        ):
            # gpsimd, in program order: warm-src memset, const iotas, then
            # the SWDGE cast-DMA issues (so the PE warm-up and constants
            # are ready before descriptor generation hogs the Q7).
            wsrc = cpool.tile([_P, 512], bf16, tag="wsrc")
            nc.gpsimd.memset(wsrc[:], 0.0)
            iot_i = cpool.tile([_P, _P], i32, tag="iot_i")
            iot_m = cpool.tile([_P, _E], i32, tag="iot_m")
            ident = cpool.tile([_P, _P], bf16, tag="ident")
            mask = cpool.tile([_P, _E], bf16, tag="mask")
            mask_b = cpool.tile([_P, _E], bf16, tag="mask_b")
            nc.gpsimd.iota(iot_i[:], pattern=[[1, _P]], base=0, channel_multiplier=-1)
            nc.gpsimd.iota(iot_m[:], pattern=[[1, _E]], base=0, channel_multiplier=-1)
            nc.vector.tensor_scalar(ident[:], iot_i[:], 0, None, op0=ALU.is_equal)
            nc.vector.tensor_scalar(mask[:], iot_m[:], 0, None, op0=ALU.is_equal)
            nc.vector.tensor_scalar(mask_b[:], iot_m[:], -_E, None, op0=ALU.is_equal)
            nc.vector.tensor_add(mask[:], mask[:], mask_b[:])

            # Warm op: forces the ACT function-table load early.
            warm = spool.tile([_P, 1], f32, tag="warm")
            nc.gpsimd.memset(warm[:], 0.0)
            nc.scalar.activation(warm[:], warm[:], SQ)

            # Input DMAs, both chunks up front.
            xbs, xsqs, sps, qts = [], [], [], []
            for c in range(n_chunks):
                rows = slice(c * _P, (c + 1) * _P)
                xb = xbpool.tile([_P, _ROW], bf16, tag="xb")
                xsq = sqpool.tile([_P, _ROW], bf16, tag="xsq")
                sp = sppool.tile([_P, 2 * _E], f32, tag="sp")
                q_t = accpool.tile([_P, _E], f32, tag="q")
                xbs.append(xb)
                xsqs.append(xsq)
                sps.append(sp)
                qts.append(q_t)
                if _CAST_DMA:
                    for c0, c1 in _SLICES:
                        nc.gpsimd.dma_start(out=xb[:, c0:c1], in_=x[rows, c0:c1])
                else:
                    xt = xtpool.tile([_P, _ROW], f32, tag="xt")
                    eng = [nc.sync, nc.scalar][c]
                    for c0, c1 in _SLICES:
                        eng.dma_start(out=xt[:, c0:c1], in_=x[rows, c0:c1])
                    for c0, c1 in _SLICES:
                        nc.vector.tensor_copy(xb[:, c0:c1], xt[:, c0:c1])

            # PE warm-up: wide dummy matmuls during the DMA fill.
            wp = wppool.tile([_P, 512], f32, tag="wp")
            for i in range(_NWARM):
                nc.tensor.matmul(wp[:], wsrc[:, 0:_P], wsrc[:], start=True, stop=True)

            # DVE s-trees per slice, both chunks.
            for c in range(n_chunks):
                for si, (c0, c1) in enumerate(_SLICES):
                    sc = trpool.tile([_P, 1472], bf16, tag="tr")
                    _emit_tree(
                        nc, xbs[c], c0,
                        sps[c][:, si * _E : (si + 1) * _E], sc,
                    )

            # PE transposes + ACT squares + q-chains + combine, software-
            # pipelined by one group so PE never stalls on ACT.
            for c in range(n_chunks):
                rows = slice(c * _P, (c + 1) * _P)
                xb, xsq, sparts = xbs[c], xsqs[c], sps[c]
                q_ps = qts[c][:]

                def emit_group_mms(g):
                    b0, b1 = _TGROUPS[g]
                    for k in range(b0, b1):
                        nc.tensor.matmul(
                            q_ps,
                            xsq[:, k * _P : (k + 1) * _P],
                            mask[:],
                            start=(k == 0),
                            stop=(k == _NBLK - 1),
                        )

                for g, (b0, b1) in enumerate(_TGROUPS):
                    gw = (b1 - b0) * _P
                    pt = ptpool.tile([_P, 5 * _P], bf16, tag="pt")
                    for j, k in enumerate(range(b0, b1)):
                        nc.tensor.transpose(
                            pt[:, j * _P : (j + 1) * _P],
                            xb[:, k * _P : (k + 1) * _P],
                            ident[:],
                        )
                    nc.scalar.activation(
                        xsq[:, b0 * _P : b1 * _P], pt[:, :gw], SQ, scale=HALF_SQRT
                    )
                    if g > 0:
                        emit_group_mms(g - 1)
                emit_group_mms(len(_TGROUPS) - 1)

                # s = spart0 + spart1; res = 0.5*s^2 - q'
                s_t = spool.tile([_P, _E], f32, tag="s")
                m2 = spool.tile([_P, _E], f32, tag="m2")
                res = spool.tile([_P, _E], f32, tag="res")
                nc.vector.tensor_add(s_t[:], sparts[:, 0:64], sparts[:, 64:128])
                nc.scalar.activation(m2[:], s_t[:], SQ, scale=HALF_SQRT)
                nc.vector.tensor_sub(res[:], m2[:], q_ps)
                nc.sync.dma_start(out=out[rows, :], in_=res[:])
    nc.compile()
    return nc


_WALRUS_EXTRA = []


def _patch_walrus():
    """Cap walrus's semaphore allocation (unused semaphores cost ~150ns
    each in the NEFF postamble)."""
    from concourse import bass_utils

    if getattr(bass_utils, "_walrus_patched", False):
        return
    real_run = bass_utils.run_command

    def run2(cmd, **kw):
        if cmd and "walrus_driver" in str(cmd[0]):
            cmd = list(cmd) + _WALRUS_EXTRA
        return real_run(cmd, **kw)

    bass_utils.run_command = run2
    bass_utils._walrus_patched = True


def _run(in_maps, **kwargs):
    from concourse.bass_utils import run_bass_kernel_spmd

    _patch_walrus()
    nc = _build()
    return run_bass_kernel_spmd(nc, in_maps, core_ids=list(range(_NCORES)), **kwargs)


def _shard(inputs: np.ndarray):
    x = np.ascontiguousarray(
        np.asarray(inputs, dtype=np.float32).reshape(_B, _ROW)
    )
    return [
        {"inputs": np.ascontiguousarray(x[i * _BS : (i + 1) * _BS])}
        for i in range(_NCORES)
    ]


def kernel(
    inputs: np.ndarray,
    weight_attention: np.ndarray = None,
    weight_projection: np.ndarray = None,
    weight_bias: np.ndarray = None,
) -> np.ndarray:
    # weights are dead code (softmax over a size-1 axis == 1.0)
    res = _run(_shard(inputs))
    return np.concatenate([r["out"] for r in res.results], axis=0)


# revision 6
# speedup vs baseline: 1.1754x; 1.1000x over previous
"""Trainium2 Bass kernel for nn_AttentionLayer_77558519431766.

Math: the reference computes softmax over a size-1 axis, which is
identically 1.0, so the attention MLP is dead code and

    out[b, e] = sum_{i<j} x[b,i,e] * x[b,j,e]
              = 0.5 * ((sum_f x[b,f,e])^2 - sum_f x[b,f,e]^2)

Design (v4, per 128-sample chunk, layout [128b, f*64+e]):
  - Input arrives via SWDGE cast-DMA (gpsimd): f32 in HBM -> bf16 in
    SBUF. Slices [24, 24, 2] fields per chunk: big slices stream, the
    tiny last slice keeps the post-stream serial tail short. DMA
    issues go first in the gpsimd program so the stream starts ASAP.
  - DVE computes s = sum_f x with an in-layout pairwise halving tree
    (5 big ops per 24-field slice; boundaries stay 64-col aligned).
  - PE transposes 128-col blocks (2 fields) into PSUM; ACT squares
    them back to SBUF with scale sqrt(0.5) (-> 0.5*x^2); PE runs a
    mask-stationary accumulation chain for q = sum_f 0.5*x^2
    (26.7ns/matmul warm - LDWEIGHTS fully pipelines).
  - res = 0.5*s^2 - q' via one ACT square + one DVE subtract.
  - Tail: drain only, no final all-engine barrier.

Sharding: pure data parallelism, batch 2048 -> 8 shards of 256.
"""

import numpy as np

try:
    import concourse.bass as bass  # noqa: F401
except ImportError:  # pragma: no cover
    import sys

    sys.path.insert(0, "/opt/trn_rl_repo")

_B, _F, _E = 2048, 50, 64
_NCORES = 8
_BS = _B // _NCORES  # 256 rows per core
_ROW = _F * _E  # 3200 floats per row
_P = 128  # SBUF partitions

_SLICES = [(0, 1536, 24), (1536, 3072, 24), (3072, 3200, 2)]
_TGROUPS = [(0, 5), (5, 10), (10, 15), (15, 20), (20, 25)]
_NBLK = 25
_NO_TAIL_BARRIER = True


def _make_tc_class():
    """TileContext with a slim kernel tail: keep only the global-clock
    drain (output DMA completion); the final all-engine barrier and sem
    clears are redundant for a single-TileContext kernel (the Bass
    preamble re-clears semaphores on every execution)."""
    from concourse.tile import TileContext
    from concourse.vector_clock import ScopedClock

    class SlimTailTileContext(TileContext):
        def _drain_and_barrier(self, tick_clock, wait_clock):
            drain_inst = self.nc.sync.drain()
            wait_clock.add_sem_waits(
                drain_inst.ins, ScopedClock({None: tick_clock.global_clock})
            )
            if not _NO_TAIL_BARRIER:
                self.nc.all_engine_barrier(sem_only=True)
            popped = self.nc._tile_sem_poison_stack.pop()
            assert popped is self._sem_poison

    return SlimTailTileContext


def _emit_tree(nc, xb, c0, nf, out_ap, sc):
    """Sum nf contiguous 64-col fields of xb starting at col c0 into
    out_ap [128, 64] via pairwise halving adds (bf16 temps in sc)."""
    add = nc.vector.tensor_add
    X = lambda a, b: xb[:, c0 + a : c0 + b]
    if nf == 2:
        add(out_ap, X(0, 64), X(64, 128))
        return
    assert nf == 24
    add(sc[:, 0:768], X(0, 768), X(768, 1536))               # 12 fields
    add(sc[:, 768:1152], sc[:, 0:384], sc[:, 384:768])       # 6
    add(sc[:, 1152:1344], sc[:, 768:960], sc[:, 960:1152])   # 3
    add(sc[:, 1344:1408], sc[:, 1152:1216], sc[:, 1216:1280])  # 1 (+left)
    add(out_ap, sc[:, 1344:1408], sc[:, 1280:1344])


def _build():
    import concourse.bacc as bacc
    import concourse.mybir as mybir

    TileContext = _make_tc_class()

    f32 = mybir.dt.float32
    bf16 = mybir.dt.bfloat16
    i32 = mybir.dt.int32
    SQ = mybir.ActivationFunctionType.Square
    ALU = mybir.AluOpType
    HALF_SQRT = float(np.float32(np.sqrt(0.5)))

    nc = bacc.Bacc()
    x = nc.declare_dram_parameter("inputs", [_BS, _ROW], f32, isOutput=False)
    out = nc.declare_dram_parameter("out", [_BS, _E], f32, isOutput=True)

    n_chunks = _BS // _P  # 2

    with TileContext(nc) as tc:
        with (
            tc.tile_pool(name="consts", bufs=1) as cpool,
            tc.tile_pool(name="xb", bufs=2) as xbpool,
            tc.tile_pool(name="xsq", bufs=2) as sqpool,
            tc.tile_pool(name="tree", bufs=2) as trpool,
            tc.tile_pool(name="sp", bufs=2) as sppool,
            tc.tile_pool(name="pt", bufs=3, space="PSUM") as ptpool,
            tc.tile_pool(name="acc", bufs=2, space="PSUM") as accpool,
            tc.tile_pool(name="small", bufs=2) as spool,
        ):
            # ACT warm op first on gpsimd/scalar: hoists the ACT
            # function-table load off the critical path.
            warm = spool.tile([_P, 1], f32, tag="warm")
            nc.gpsimd.memset(warm[:], 0.0)
            nc.scalar.activation(warm[:], warm[:], SQ)

            # Tiles for both chunks.
            xbs, xsqs, sps, qts = [], [], [], []
            for c in range(n_chunks):
                xb = xbpool.tile([_P, _ROW], bf16, tag="xb")
                xsq = sqpool.tile([_P, _ROW], bf16, tag="xsq")
                sp = sppool.tile([_P, 3 * _E], f32, tag="sp")
                q_t = accpool.tile([_P, _E], f32, tag="q")
                xbs.append(xb)
                xsqs.append(xsq)
                sps.append(sp)
                qts.append(q_t)

            # Chunk 0's big slices stream first...
            rows0 = slice(0, _P)
            for c0, c1, nf in _SLICES[:2]:
                nc.gpsimd.dma_start(out=xbs[0][:, c0:c1], in_=x[rows0, c0:c1])

            # ...then constants (identity + stacked two-hot mask) while
            # the first slice is in flight...
            iot_i = cpool.tile([_P, _P], i32, tag="iot_i")
            iot_m = cpool.tile([_P, _E], i32, tag="iot_m")
            ident = cpool.tile([_P, _P], bf16, tag="ident")
            mask = cpool.tile([_P, _E], bf16, tag="mask")
            mask_b = cpool.tile([_P, _E], bf16, tag="mask_b")
            nc.gpsimd.iota(iot_i[:], pattern=[[1, _P]], base=0, channel_multiplier=-1)
            nc.gpsimd.iota(iot_m[:], pattern=[[1, _E]], base=0, channel_multiplier=-1)
            nc.vector.tensor_scalar(ident[:], iot_i[:], 0, None, op0=ALU.is_equal)
            nc.vector.tensor_scalar(mask[:], iot_m[:], 0, None, op0=ALU.is_equal)
            nc.vector.tensor_scalar(mask_b[:], iot_m[:], -_E, None, op0=ALU.is_equal)
            nc.vector.tensor_add(mask[:], mask[:], mask_b[:])

            # ...then the remaining input DMAs.
            rows1 = slice(_P, 2 * _P)
            nc.gpsimd.dma_start(
                out=xbs[0][:, 3072:3200], in_=x[rows0, 3072:3200]
            )
            for c0, c1, nf in _SLICES:
                nc.gpsimd.dma_start(out=xbs[1][:, c0:c1], in_=x[rows1, c0:c1])

            # DVE s-trees per slice, both chunks; pre-add the two big
            # partials so the tail only needs one small add.
            p01s = []
            for c in range(n_chunks):
                for si, (c0, c1, nf) in enumerate(_SLICES):
                    sc = trpool.tile([_P, 1408], bf16, tag="tr")
                    _emit_tree(
                        nc, xbs[c], c0, nf,
                        sps[c][:, si * _E : (si + 1) * _E], sc,
                    )
                p01 = spool.tile([_P, _E], f32, tag="p01")
                nc.vector.tensor_add(p01[:], sps[c][:, 0:64], sps[c][:, 64:128])
                p01s.append(p01)

            # PE transposes + ACT squares + q-chains + combine, software-
            # pipelined by one group so PE never stalls on ACT.
            for c in range(n_chunks):
                rows = slice(c * _P, (c + 1) * _P)
                xb, xsq, sparts = xbs[c], xsqs[c], sps[c]
                q_ps = qts[c][:]

                def emit_group_mms(g):
                    b0, b1 = _TGROUPS[g]
                    for k in range(b0, b1):
                        nc.tensor.matmul(
                            q_ps,
                            xsq[:, k * _P : (k + 1) * _P],
                            mask[:],
                            start=(k == 0),
                            stop=(k == _NBLK - 1),
                        )

                for g, (b0, b1) in enumerate(_TGROUPS):
                    gw = (b1 - b0) * _P
                    pt = ptpool.tile([_P, 5 * _P], bf16, tag="pt")
                    for j, k in enumerate(range(b0, b1)):
                        nc.tensor.transpose(
                            pt[:, j * _P : (j + 1) * _P],
                            xb[:, k * _P : (k + 1) * _P],
                            ident[:],
                        )
                    nc.scalar.activation(
                        xsq[:, b0 * _P : b1 * _P], pt[:, :gw], SQ, scale=HALF_SQRT
                    )
                    if g > 0:
                        emit_group_mms(g - 1)
                emit_group_mms(len(_TGROUPS) - 1)

                # s = p01 + p2; res = 0.5*s^2 - q'
                s_t = spool.tile([_P, _E], f32, tag="s")
                m2 = spool.tile([_P, _E], f32, tag="m2")
                res = spool.tile([_P, _E], f32, tag="res")
                nc.vector.tensor_add(s_t[:], p01s[c][:], sparts[:, 128:192])
                nc.scalar.activation(m2[:], s_t[:], SQ, scale=HALF_SQRT)
                nc.vector.tensor_sub(res[:], m2[:], q_ps)
                eng = nc.sync if c == 0 else nc.scalar
                eng.dma_start(out=out[rows, :], in_=res[:])
    nc.compile()
    return nc


_WALRUS_EXTRA = []


def _patch_walrus():
    """Cap walrus's semaphore allocation (unused semaphores cost ~150ns
    each in the NEFF postamble)."""
    from concourse import bass_utils

    if getattr(bass_utils, "_walrus_patched", False):
        return
    real_run = bass_utils.run_command

    def run2(cmd, **kw):
        if cmd and "walrus_driver" in str(cmd[0]):
            cmd = list(cmd) + _WALRUS_EXTRA
        return real_run(cmd, **kw)

    bass_utils.run_command = run2
    bass_utils._walrus_patched = True


def _run(in_maps, **kwargs):
    from concourse.bass_utils import run_bass_kernel_spmd

    _patch_walrus()
    nc = _build()
    return run_bass_kernel_spmd(nc, in_maps, core_ids=list(range(_NCORES)), **kwargs)


def _shard(inputs: np.ndarray):
    x = np.ascontiguousarray(
        np.asarray(inputs, dtype=np.float32).reshape(_B, _ROW)
    )
    return [
        {"inputs": np.ascontiguousarray(x[i * _BS : (i + 1) * _BS])}
        for i in range(_NCORES)
    ]


def kernel(
    inputs: np.ndarray,
    weight_attention: np.ndarray = None,
    weight_projection: np.ndarray = None,
    weight_bias: np.ndarray = None,
) -> np.ndarray:
    # weights are dead code (softmax over a size-1 axis == 1.0)
    res = _run(_shard(inputs))
    return np.concatenate([r["out"] for r in res.results], axis=0)
